# revision 43
# baseline (speedup 1.0000x reference)
"""Trainium2 Bass kernel for MembraneNet (PINN forward + analytic PDE residual).

Math (per collocation point): 4-layer tanh MLP u(x,y); PDE = K*(uxx+uyy)
+ Kx*ux + Ky*uy + f. Forward-mode propagation of (h, gx, gy, lap) per layer,
O(H^2)/point. Batch sharded 8 ways (2048 points/core); on each core points
sit in two 64-feature chunks on SBUF partitions 0-63/64-127 with
block-diagonal weights, 1024 columns per stream.

v4 (~46.5-47us vs v2 ~48.5-49.7us; measured floor components: ~1.5us NEFF
preamble + ~2us DMA-to-first-matmul + compute + ~2us tile-exit drains +
~6.4us harness semaphore-reset tail that scales with nothing we control):
- Compact inputs (~210KB vs 568KB): no zero-padded 128-row xy/W0 (L0 runs as
  a 4-contraction matmul). The lap carrier sign alternates per layer
  (sigma = -,+,-,+) so pp = W@c + (sigma I)@t with only +W weights; +I/-I
  both shipped. The L0 lap seed's -2*q0 scale is folded into a pre-scaled
  W1 copy (WLP1) so the seed is a single stt into the gfl slot.
- L0 operands ride the sync HWDGE queue alone; the weight pack rides the
  scalar queue behind the ACT-table warmup; epilogue tensors trail on sync.
- PE-HAM management: warmup matmuls on zeroed SBUF before/after L0 plus
  dependency-free fills (psum regions whose last reader retires early), so
  the 1.2GHz->2.4GHz clock gate warms early and gaps stay under the ~3.4us
  idle window. The HAM state is worth ~250ns per 512-col matmul.
- h^2 on the Pool engine (tanh->hsq->gxy stt is the pacing chain; ACT was
  the busier engine). q2/t per block on DVE 2x. gxy/lambda stts are 1x
  (PSUM operand) and dominate DVE; DVE is the end-to-end pacer.
- Epilogue without the [2,F]->[128,FT] DMA-transpose roundtrip: paired
  reductions land at PSUM partition bases 0 and 64 in SEPARATE tiles (Tile's
  range tracker is partition-blind within a tile -> false WAR serialization
  otherwise), fp16 coefficient mults (1,Kx | Ky,K), and two accumulating
  [*,4] fp16 assembly matmuls produce [u_A,u_B,pde_A,pde_B] rows; assembly
  matmuls are emitted after the S reductions so the in-order PE queue never
  head-blocks; ACT-copy + straight [2,F]-layout DMAs out, per block.
"""

import sys

sys.path.insert(0, "/opt/trn_rl_repo")

import numpy as np
from contextlib import ExitStack

import concourse.bass as bass
import concourse.mybir as mybir
import concourse.tile as tile

B = 16384
H = 64
L = 4
NCORES = 8
BC = B // NCORES          # 2048 points per core
F = BC // 2               # 1024 columns (2 chunks of 1024 points on partitions)

f32 = mybir.dt.float32
fp16 = mybir.dt.float16
bf16 = mybir.dt.bfloat16
AF = mybir.ActivationFunctionType
OP = mybir.AluOpType

SQRT2 = float(np.sqrt(2.0))

# lap-carrier sign per layer: c_k = sigma_k * lap_k, sigma_0 = -1 (L0 seed is
# -lap_0); with only +W available sigma alternates and t~_k = sigma_k * t_k.
SIGMA = [-1.0, 1.0, -1.0, 1.0]

# xyw bf16 [4, 128 + F]: L0 lhsT then xy rows
XW_W0T = 0
XW_XY = 128
XW_COLS = 128 + F

# wpack bf16 [128, 776]
WP_WT = [None, 0, 128, 256]
WP_I = 384                 # +I128 (t add, layers 1,3)
WP_NI = 512                # -I128 (t add, layer 2: sigma = -1)
WP_LP1 = 640               # W1.T with rows scaled by -2*q0 (L0 lap seed)
WP_WL = 768                # WLa [128,4] (Wout in cols 0,1), WLb (cols 2,3)
WP_COLS = 776

# cpack f32 [128, 8]: b0..b3, -2*q0, w0x, w0y, (pad)
CP_B = [0, 1, 2, 3]
CP_M2Q0 = 4
CP_W0X = 5
CP_W0Y = 6
CP_COLS = 8

# kpack fp16 [8, 2F + 8]: rows 0-3 = mult coeffs (1,1,Kx_A,Kx_B) in cols 0:F
# and (Ky_A,Ky_B,K_A,K_B) in cols F:2F; rows 4-7 cols 0:F = (f_A,f_B,1,1);
# cols 2F:2F+8 = assembly lhsTs A (4x4) and B (8x4).
KP_COLS = 2 * F + 8


def _legalize_sync_waits(bj: bytes) -> bytes:
    """The walrus in this container accepts at most ONE on_wait per
    instruction, but Tile emits several. Move excess waits into standalone
    EventSemaphore instructions right before the owner (same engine, so the
    sequencer executes them first) — the exact encoding raw-bass wait_ge uses.
    """
    import json

    m = json.loads(bj)
    n = 0
    for fn in m.get("functions", []):
        for blk in fn.get("blocks", []):
            out = []
            for ins in blk.get("instructions", []):
                si = ins.get("sync_info") or {}
                waits = si.get("on_wait") or []
                if len(waits) > 1:
                    for w in waits[:-1]:
                        n += 1
                        out.append(
                            {
                                "name": f"lsw_{n}",
                                "opcode": "EventSemaphore",
                                "engine": ins["engine"],
                                "ins": [],
                                "outs": [],
                                "debug": ins.get("debug", 0),
                                "sync_info": {"on_update": [], "on_wait": [w]},
                            }
                        )
                    si["on_wait"] = waits[-1:]
                out.append(ins)
            blk["instructions"] = out
    return json.dumps(m).encode()


def build_nc():
    nc = bass.Bass()

    # ---- I/O (everything preformatted on host) ----
    xyw_d = nc.dram_tensor("xyw", [4, XW_COLS], bf16, kind="ExternalInput")
    wpack_d = nc.dram_tensor("wpack", [128, WP_COLS], bf16, kind="ExternalInput")
    cpack_d = nc.dram_tensor("cpack", [128, CP_COLS], f32, kind="ExternalInput")
    kpack_d = nc.dram_tensor("kpack", [8, KP_COLS], fp16, kind="ExternalInput")
    u_d = nc.dram_tensor("u", [BC], f32, kind="ExternalOutput")
    pde_d = nc.dram_tensor("pde", [BC], f32, kind="ExternalOutput")

    NB = 2              # column blocks per core (pipelined chains)
    FB = F // NB        # 512 columns per block

    with tile.TileContext(nc) as tc, ExitStack() as ctx:
        const = ctx.enter_context(tc.tile_pool(name="const", bufs=1))
        sb = ctx.enter_context(tc.tile_pool(name="sb", bufs=3))
        ps = ctx.enter_context(tc.tile_pool(name="ps", bufs=1, space="PSUM"))

        # ---- warmup constants (emitted first so engines start instantly) ----
        wrm = const.tile([1, 1], f32, tag="wrm")
        nc.vector.memset(wrm[:], 0.0)
        warmR = const.tile([128, F], bf16, tag="warmR")
        nc.vector.memset(warmR[:], 0.0)
        warm_rhs = warmR[:]

        # ---- input DMAs, spread across HWDGE queues ----
        xyw = const.tile([4, XW_COLS], bf16, tag="xyw")
        cpack = const.tile([128, CP_COLS], f32, tag="cpack")
        wpack = const.tile([128, WP_COLS], bf16, tag="wpack")
        kmult = const.tile([4, 2 * F], fp16, tag="kmult")
        prods_a = const.tile([4, F], fp16, tag="prods_a")
        prods_b = const.tile([8, F], fp16, tag="prods_b")
        finl = const.tile([8, 8], fp16, tag="finl")

        # sync HWDGE queue: the L0 operands alone (shortest path to compute),
        # then the late-needed epilogue tensors
        nc.sync.dma_start(out=xyw[:], in_=xyw_d[:, :])
        nc.sync.dma_start(out=cpack[:], in_=cpack_d[:, :])
        nc.sync.dma_start(out=kmult[:], in_=kpack_d[0:4, 0 : 2 * F])
        nc.sync.dma_start(out=prods_b[4:8, :], in_=kpack_d[4:8, 0:F])
        nc.sync.dma_start(out=finl[:], in_=kpack_d[:, 2 * F : 2 * F + 8])

        # ---- ACT table warmup (hide the ~2.7us table load under DMA wait),
        # then the weight pack rides the scalar HWDGE queue ----
        nc.scalar.activation(wrm[:], wrm[:], AF.Tanh)
        nc.scalar.dma_start(out=wpack[:, 0:128], in_=wpack_d[:, 0:128])
        nc.scalar.dma_start(out=wpack[:, 128:384], in_=wpack_d[:, 128:384])
        nc.scalar.dma_start(out=wpack[:, 384:WP_COLS], in_=wpack_d[:, 384:WP_COLS])

        WT = [None] + [wpack[:, WP_WT[k] : WP_WT[k] + 128] for k in (1, 2, 3)]
        I128 = wpack[:, WP_I : WP_I + 128]
        NI128 = wpack[:, WP_NI : WP_NI + 128]
        WLP1 = wpack[:, WP_LP1 : WP_LP1 + 128]
        WLa = wpack[:, WP_WL : WP_WL + 4]
        WLb = wpack[:, WP_WL + 4 : WP_WL + 8]
        W0T = xyw[0:4, XW_W0T : XW_W0T + 128]
        xyrows = xyw[0:4, XW_XY : XW_XY + F]
        bcol = [cpack[:, k : k + 1] for k in CP_B]
        m2q0 = cpack[:, CP_M2Q0 : CP_M2Q0 + 1]
        w0x = cpack[:, CP_W0X : CP_W0X + 1]
        w0y = cpack[:, CP_W0Y : CP_W0Y + 1]

        def BS(b):  # block slice within an F-wide region
            return slice(b * FB, (b + 1) * FB)

        def mm(out, lhsT, rhs, start=True, stop=True):
            nc.tensor.matmul(out, lhsT, rhs, start=start, stop=stop)

        def warm_fill(n):
            # Filler matmuls into the zp-region: its last reader (tanh)
            # finishes early each layer, so these run inside the PE's
            # dependency gaps and keep the HAM clock at 2.4 GHz. A burst
            # shorter than ~3.4us of PE idle never re-warms the clock, so
            # bridging the gaps is what keeps the whole kernel fast.
            wtile = ps.tile([128, F], f32, tag="zp")
            for i in range(n):
                mm(wtile[:, BS(i % 2)], warmR[:, 0:128], warmR[:, 0:512])

        # ---- PE/HAM warmup matmul on zeroed SBUF during the DMA wait ----
        warm_out = ps.tile([128, F], f32, tag="zp")
        mm(warm_out[:, BS(0)], warmR[:, 0:128], warmR[:, 0:512])

        # ---- layer 0 (4-contraction matmul on the raw xy rows) ----
        zp = ps.tile([128, F], f32, tag="zp")
        h = sb.tile([128, F], bf16, tag="h")
        hsq = sb.tile([128, F], bf16, tag="hsq")
        m = sb.tile([128, F], bf16, tag="m")
        gfl = sb.tile([128, 3 * F], bf16, tag="gfl")
        for b in range(NB):
            mm(zp[:, BS(b)], W0T, xyrows[:, BS(b)])
        # dependency-free warmups bridge the PE gap until L1 is ready, so the
        # HAM activity window never sees an idle reset during startup
        warm_out2 = ps.tile([128, F], f32, tag="zxy")
        for i in range(2):
            mm(warm_out2[:, BS(i % 2)], warmR[:, 0:128], warmR[:, 0:512])
        warm_out3 = ps.tile([128, F], f32, tag="pp")
        for i in range(3):
            mm(warm_out3[:, BS(i % 2)], warmR[:, 0:128], warmR[:, 0:512])
        for b in range(NB):
            s = BS(b)
            nc.scalar.activation(h[:, s], zp[:, s], AF.Tanh, bias=bcol[0], scale=1.0)
            nc.scalar.activation(hsq[:, s], h[:, s], AF.Square)
            nc.vector.tensor_scalar(
                gfl[:, b * FB : b * FB + FB], hsq[:, s], -1.0, w0x,
                OP.add, OP.mult,
            )
            nc.vector.tensor_scalar(
                gfl[:, F + b * FB : F + b * FB + FB], hsq[:, s], -1.0, w0y,
                OP.add, OP.mult,
            )
            # lap seed (hsq-1)*h straight into the gfl lap slot; the -2*q0
            # scale is folded into WLP1 on the host
            nc.vector.scalar_tensor_tensor(
                gfl[:, 2 * F + b * FB : 2 * F + b * FB + FB],
                hsq[:, s], -1.0, h[:, s], OP.add, OP.mult,
            )

        # ---- layers 1..3: software-pipelined. Stage A(k) = h/gxy chain
        # (self-contained across layers); stage B(k) = lap tail. Emit order
        # A1 A2 B1 A3 B2 B3 so B's matmuls slot into queue gaps. ----
        def gx_s(b):
            return slice(b * FB, b * FB + FB)

        def gy_s(b):
            return slice(F + b * FB, F + b * FB + FB)

        def lp_s(b):
            return slice(2 * F + b * FB, 2 * F + b * FB + FB)

        def stage_A(k, hP, gflP):
            st = {}
            hN = sb.tile([128, F], bf16, tag="h", name="hN")
            st["h"] = hN
            hsqN = sb.tile([128, F], bf16, tag="hsq", name="hsqN")
            st["hsq"] = hsqN
            sqs = sb.tile([128, 2 * F], bf16, tag="sqs")
            q2 = sb.tile([128, F], bf16, tag="q2")
            t = sb.tile([128, F], bf16, tag="t", name="tN")
            st["t"] = t
            zxyN = ps.tile([128, 2 * F], f32, tag="zxy", name="zxyN")
            st["zxy"] = zxyN
            zpN = ps.tile([128, F], f32, tag="zp")
            gflN = sb.tile([128, 3 * F], bf16, tag="gfl", name="gflN")
            st["gfl"] = gflN
            for b in range(NB):
                mm(zpN[:, BS(b)], WT[k], hP[:, BS(b)])
            for b in range(NB):
                mm(zxyN[:, gx_s(b)], WT[k], gflP[:, gx_s(b)])
                mm(zxyN[:, gy_s(b)], WT[k], gflP[:, gy_s(b)])
            warm_fill(2)
            for b in range(NB):
                # squares land x-half at sqs[0:F], y-half at sqs[F:2F] so the
                # q2 add runs F-wide at DVE 2x
                sq_view = bass.AP(
                    tensor=sqs[:].tensor,
                    offset=sqs[:].offset + b * FB,
                    ap=[[sqs[:].ap[0][0], 128], [F, 2], [1, FB]],
                )
                zxy_view = bass.AP(
                    tensor=zxyN[:].tensor,
                    offset=zxyN[:].offset + b * FB,
                    ap=[[zxyN[:].ap[0][0], 128], [F, 2], [1, FB]],
                )
                nc.scalar.activation(sq_view, zxy_view, AF.Square, scale=SQRT2)
                nc.scalar.activation(
                    hN[:, BS(b)], zpN[:, BS(b)], AF.Tanh, bias=bcol[k], scale=1.0
                )
                # h^2 on the Pool engine: unloads ACT, latency is off-pace
                nc.gpsimd.tensor_mul(hsqN[:, BS(b)], hN[:, BS(b)], hN[:, BS(b)])
            for b in range(NB):
                s = BS(b)
                nc.vector.tensor_add(
                    q2[:, s], sqs[:, s], sqs[:, F + b * FB : F + b * FB + FB]
                )
                nc.vector.tensor_mul(t[:, s], hN[:, s], q2[:, s])
            for b in range(NB):
                hsq_rep = bass.AP(
                    tensor=hsqN[:].tensor,
                    offset=hsqN[:].offset + b * FB,
                    ap=[[hsqN[:].ap[0][0], 128], [0, 2], [1, FB]],
                )
                zxy2 = bass.AP(
                    tensor=zxyN[:].tensor,
                    offset=zxyN[:].offset + b * FB,
                    ap=[[zxyN[:].ap[0][0], 128], [F, 2], [1, FB]],
                )
                gxy_out = bass.AP(
                    tensor=gflN[:].tensor,
                    offset=gflN[:].offset + b * FB,
                    ap=[[gflN[:].ap[0][0], 128], [F, 2], [1, FB]],
                )
                nc.vector.scalar_tensor_tensor(
                    gxy_out, hsq_rep, -1.0, zxy2, OP.add, OP.mult
                )
            return st

        def stage_B(k, gflP, st):
            # pp = W@c + (sigma*I)@t
            ppN = ps.tile([128, F], f32, tag="pp")
            ident = I128 if SIGMA[k] > 0 else NI128
            wlap = WLP1 if k == 1 else WT[k]
            for b in range(NB):
                mm(ppN[:, BS(b)], wlap, gflP[:, lp_s(b)], start=True, stop=False)
            for b in range(NB):
                mm(ppN[:, BS(b)], ident, st["t"][:, BS(b)], start=False, stop=True)
            if k == 3:
                for b in range(NB):
                    nc.vector.scalar_tensor_tensor(
                        st["gfl"][:, lp_s(b)], st["hsq"][:, BS(b)], -1.0,
                        ppN[:, BS(b)], OP.add, OP.mult,
                    )
            else:
                nc.vector.scalar_tensor_tensor(
                    st["gfl"][:, 2 * F : 3 * F], st["hsq"][:], -1.0,
                    ppN[:], OP.add, OP.mult,
                )
            warm_fill(1)

        S1 = stage_A(1, h, gfl)
        S2 = stage_A(2, S1["h"], S1["gfl"])
        stage_B(1, gfl, S1)
        S3 = stage_A(3, S2["h"], S2["gfl"])
        stage_B(2, S1["gfl"], S2)
        h3, gfl3 = S3["h"], S3["gfl"]

        # ---- output reductions: matmul out base partitions must be 0/32/64,
        # so (u,ux) stack at PSUM rows 0-3 and (uy,S) at 64-67, each pair via
        # two complementary zero-padded lhsTs sharing one accumulation group.
        # The early (u,ux) half gets its fp16 coefficient mult under the lap
        # tail ----
        # Separate PSUM tags per reduction group: Tile's range tracking is
        # partition-blind within a tile, so sharing one tile creates false
        # WAR edges that serialize the tail.
        ps_red_a = ps.tile([4, F], f32, tag="zp")
        ps_red_b = ps.tile([68, F], f32, tag="zxy")
        for b in range(NB):
            s = BS(b)
            mm(ps_red_a[:, s], WLa, h3[:, s], start=True, stop=False)
            mm(ps_red_b[64:68, s], WLa, gfl3[:, gy_s(b)], start=True, stop=False)
        for b in range(NB):
            mm(ps_red_a[:, BS(b)], WLb, gfl3[:, gx_s(b)], start=False, stop=True)
        nc.vector.tensor_mul(prods_a[:], ps_red_a[:], kmult[:, 0:F])

        stage_B(3, S2["gfl"], S3)

        # ---- tail: S reduction closes the (uy,S) group; assembly matmuls are
        # emitted after the S matmuls so the in-order PE queue never stalls ----
        pde_u = ps.tile([4, F], f32, tag="pp")
        fin4 = sb.tile([4, F], f32, tag="fin4")
        u_view = u_d[:].rearrange("(r c) -> r c", r=2)
        pde_view = pde_d[:].rearrange("(r c) -> r c", r=2)
        for b in range(NB):
            s = BS(b)
            mm(ps_red_b[64:68, s], WLb, gfl3[:, lp_s(b)], start=False, stop=True)
            nc.vector.tensor_mul(
                prods_b[0:4, s], ps_red_b[64:68, s],
                kmult[:, F + b * FB : F + b * FB + FB],
            )
            mm(pde_u[:, s], finl[0:4, 0:4], prods_a[:, s], start=True, stop=False)
            mm(pde_u[:, s], finl[:, 4:8], prods_b[:, s], start=False, stop=True)
            nc.scalar.activation(fin4[:, s], pde_u[:, s], AF.Copy)
            nc.sync.dma_start(out=u_view[:, s], in_=fin4[0:2, s])
            nc.sync.dma_start(out=pde_view[:, s], in_=fin4[2:4, s])

    if not nc.is_finalized():
        nc.finalize()
    legalized = _legalize_sync_waits(nc.to_json_bytes())
    nc.to_json_bytes = lambda: legalized
    return nc


_NC = None


def _get_nc():
    global _NC
    if _NC is None:
        _NC = build_nc()
    return _NC


def _host_prep(full):
    """Build the shared (weight/const) arrays once (xy appended per core)."""
    import ml_dtypes

    b16 = ml_dtypes.bfloat16
    W = [full[f"W{i}"] for i in range(L)]
    bvec = [full[f"b{i}"] for i in range(L)]
    Wout = full["Wout"]
    bout = float(full["bout"])

    wpack = np.zeros((128, WP_COLS), np.float32)
    for k in (1, 2, 3):
        wt = W[k].T  # [in, out] = Wk.T so lhsT.T @ rhs = Wk @ rhs
        wpack[0:H, WP_WT[k] : WP_WT[k] + H] = wt
        wpack[H:128, WP_WT[k] + H : WP_WT[k] + 128] = wt
    q0 = W[0][:, 0] ** 2 + W[0][:, 1] ** 2
    wt1s = W[1].T * (-2.0 * q0)[:, None]  # rows of W1.T scaled by -2*q0
    wpack[0:H, WP_LP1 : WP_LP1 + H] = wt1s
    wpack[H:128, WP_LP1 + H : WP_LP1 + 128] = wt1s
    wpack[:, WP_I : WP_I + 128] = np.eye(128, dtype=np.float32)
    wpack[:, WP_NI : WP_NI + 128] = -np.eye(128, dtype=np.float32)
    # reduction lhsTs: WLa puts (chunkA, chunkB) in cols 0,1; WLb in cols 2,3
    wpack[0:H, WP_WL + 0] = Wout
    wpack[H:128, WP_WL + 1] = Wout
    wpack[0:H, WP_WL + 4 + 2] = Wout
    wpack[H:128, WP_WL + 4 + 3] = Wout
    wpack = wpack.astype(b16)

    cpack = np.zeros((128, CP_COLS), np.float32)
    for k in range(L):
        cpack[0:H, CP_B[k]] = bvec[k]
        cpack[H:128, CP_B[k]] = bvec[k]
    cpack[0:H, CP_M2Q0] = -2.0 * q0
    cpack[H:128, CP_M2Q0] = -2.0 * q0
    cpack[0:H, CP_W0X] = W[0][:, 0]
    cpack[H:128, CP_W0X] = W[0][:, 0]
    cpack[0:H, CP_W0Y] = W[0][:, 1]
    cpack[H:128, CP_W0Y] = W[0][:, 1]

    # xyw weight part (xy rows filled per core)
    xyw_base = np.zeros((4, XW_COLS), np.float32)
    xyw_base[0, XW_W0T : XW_W0T + H] = W[0][:, 0]
    xyw_base[1, XW_W0T : XW_W0T + H] = W[0][:, 1]
    xyw_base[2, XW_W0T + H : XW_W0T + 128] = W[0][:, 0]
    xyw_base[3, XW_W0T + H : XW_W0T + 128] = W[0][:, 1]

    # assembly lhsT A [4, 4] over prods_a rows (u_A,u_B,Kx*ux_A,Kx*ux_B) and
    # B [8, 4] over prods_b rows (Ky*uy_A,Ky*uy_B,K*S_A,K*S_B,f_A,f_B,1_A,1_B);
    # output cols = u_A, u_B, pde_A, pde_B
    finl = np.zeros((8, 8), np.float32)
    finl[0, 0] = 1.0
    finl[1, 1] = 1.0
    finl[2, 2] = 1.0
    finl[3, 3] = 1.0
    finl[6, 4] = bout
    finl[7, 5] = bout
    finl[0, 6] = finl[2, 6] = finl[4, 6] = 1.0
    finl[1, 7] = finl[3, 7] = finl[5, 7] = 1.0

    return wpack, cpack, xyw_base, finl, b16


def make_in_maps(inputs):
    full = {k: np.asarray(v, dtype=np.float32) for k, v in inputs.items()}
    wpack, cpack, xyw_base, finl, b16 = _host_prep(full)
    in_maps = []
    for c in range(NCORES):
        s = slice(c * BC, (c + 1) * BC)
        xy = full["xy"][s]
        xyw = xyw_base.copy()
        xyw[0, XW_XY : XW_XY + F] = xy[0:F, 0]
        xyw[1, XW_XY : XW_XY + F] = xy[0:F, 1]
        xyw[2, XW_XY : XW_XY + F] = xy[F:BC, 0]
        xyw[3, XW_XY : XW_XY + F] = xy[F:BC, 1]
        kpack = np.zeros((8, KP_COLS), np.float32)
        kpack[0:2, 0:F] = 1.0
        kpack[2, 0:F] = full["Kx"][s][0:F]
        kpack[3, 0:F] = full["Kx"][s][F:BC]
        kpack[0, F : 2 * F] = full["Ky"][s][0:F]
        kpack[1, F : 2 * F] = full["Ky"][s][F:BC]
        kpack[2, F : 2 * F] = full["K"][s][0:F]
        kpack[3, F : 2 * F] = full["K"][s][F:BC]
        kpack[4, 0:F] = full["f"][s][0:F]
        kpack[5, 0:F] = full["f"][s][F:BC]
        kpack[6:8, 0:F] = 1.0
        kpack[:, 2 * F : 2 * F + 8] = finl
        in_maps.append(
            {
                "xyw": xyw.astype(b16),
                "wpack": wpack,
                "cpack": cpack,
                "kpack": kpack.astype(np.float16),
            }
        )
    return in_maps


def run(inputs, trace=False, **kw):
    from concourse.bass_utils import run_bass_kernel_spmd

    nc = _get_nc()
    res = run_bass_kernel_spmd(
        nc, make_in_maps(inputs), list(range(NCORES)), trace=trace, **kw
    )
    u = np.concatenate([r["u"] for r in res.results])
    pde = np.concatenate([r["pde"] for r in res.results])
    return (u, pde), res


def kernel(**inputs):
    (u, pde), _ = run(inputs)
    return u, pde
